# revision 1
# baseline (speedup 1.0000x reference)
"""Trainium2 Bass kernel for nn_BAC_15152644620305 (v2).

Per batch element (1 per NeuronCore, 8 cores):
  p_dense = relu(p @ W1 + b1); q_dense = relu(q @ W2 + b2)
  A = (p_dense @ q_dense.T) / sqrt(600)
  passage_aligned = softmax_rows(A) @ passage ; query_aligned = softmax_cols(A).T @ query
  6 factorization-machine heads on {concat, diff, mul} pairs -> [L, 3] x 2 outputs.

v2 implementation notes (cost-model-driven):
  - Everything that tolerates fp8 runs as fp8 DoubleRow matmuls (0.5 cyc/row):
    dense (W and xT d-paired), both affinity layouts (u-paired, incl. the
    44-row tail), aligned (nats/E as before), and the FM square/mul planes
    (x^2,b^2) and (xb,xb^2) paired as DR row-pairs with disjoint stationary
    column groups.
  - Precision-critical FM paths stay bf16: the x/b linear+quad projections
    (fp8 there costs 3e-2 rel err) and the u = sum_k V_k^2 stationaries are
    pre-scaled x256 into fp8 normal range (denormals cost 2.5e-2 otherwise),
    with the 1/256 folded into the f32->bf16 combine matrix.
  - xb^2 = x^2 * b^2 (product of already-built fp8 planes), no extra square.
  - FM combine: 4 matmul groups per PSUM tile at 32-aligned offsets with
    zero-padded stationaries so the whole [128, L] block evicts in 4 copies,
    then TA-add + 2 squares + one [128->3] combine matmul per side.
  - exp at N=1024 (two ACT ops per l-tile); PSUM: 2x[128,1024] "acc" ring +
    4x[128,512] "fmp" ring.
  - nats junk columns get narrow memsets only; elementwise planes spread
    across DVE/ACT/Pool (scalar_tensor_tensor on gpsimd).
"""
import numpy as np

L_FULL = 2048
D = 600
U = 300
KFM = 5
N_CORES = 8
SCALE = float(1.0 / np.sqrt(np.float32(D)))
USC = 256.0           # fp8 pre-scale for the u = sum V^2 stationaries

DCH = [(0, 128), (128, 128), (256, 128), (384, 128), (512, 88)]   # D chunks
ONES_COL = 96         # ones column within the 128-wide natural tail tile
ONES_ROW = 96         # denominator row in the pass-A psum
NATW = 640


def _emit(nc, L):
    import concourse.bass as bass
    import concourse.mybir as mybir
    import concourse.tile as tile
    from concourse.masks import make_identity
    from contextlib import ExitStack

    f32 = mybir.dt.float32
    bf16 = mybir.dt.bfloat16
    fp8 = mybir.dt.float8e4
    AF = mybir.ActivationFunctionType
    ALU = mybir.AluOpType
    ds = bass.ds
    DR = mybir.MatmulPerfMode.DoubleRow

    LT = L // 128               # 16 l-tiles
    NP = LT // 2                # 8 pair tiles
    TG = 4                      # l-tiles per transpose group
    NG = LT // TG               # 4 groups (512 cols each)
    GW = TG * 128               # 512
    NH = L // 1024              # halves (2)

    x_d = nc.dram_tensor("x", [2, L, D], bf16, kind="ExternalInput")
    wp_d = nc.dram_tensor("wpair", [3, 128, 2, 1024], f32, kind="ExternalInput")
    pa_d = nc.dram_tensor("pastat", [10, 128, 64], f32, kind="ExternalInput")
    pbc_d = nc.dram_tensor("pbcstat", [10, 128, 2, 64], f32, kind="ExternalInput")
    c2_d = nc.dram_tensor("comb2", [128, 6], f32, kind="ExternalInput")
    bp_d = nc.dram_tensor("biasp", [128, 6], f32, kind="ExternalInput")
    w0_d = nc.dram_tensor("w0col", [3, 2], f32, kind="ExternalInput")
    out_d = nc.dram_tensor("out", [2, 3, L], f32, kind="ExternalOutput")

    with tile.TileContext(nc) as tc, ExitStack() as ctx:
        const = ctx.enter_context(tc.tile_pool(name="const", bufs=1))
        big = ctx.enter_context(tc.tile_pool(name="big", bufs=1))
        natp = ctx.enter_context(tc.tile_pool(name="natp", bufs=LT))
        ps = ctx.enter_context(tc.tile_pool(name="ps", bufs=1, space="PSUM"))
        # scoped pools (right side), freed mid-kernel:
        dpool_cm = tc.tile_pool(name="dpool", bufs=1, side="right")
        dpool = dpool_cm.__enter__()
        xtp_cm = tc.tile_pool(name="xtp", bufs=1, side="right")
        xtpp = xtp_cm.__enter__()
        nf32_cm = tc.tile_pool(name="nf32p", bufs=3, side="right")
        nf32p = nf32_cm.__enter__()
        stg_cm = tc.tile_pool(name="stg", bufs=1, side="right")
        stg = stg_cm.__enter__()

        def acc_t(name="acc"):
            return ps.tile([128, 1024], f32, tag="acc", name=name, bufs=2)

        def fmp_t(name="fmp"):
            return ps.tile([128, 512], f32, tag="fmp", name=name, bufs=4)

        # ---------------- constants ----------------
        identb = const.tile([128, 128], bf16, tag="identb")
        make_identity(nc, identb)
        onesb = const.tile([128, 128], bf16, tag="onesb")
        nc.vector.memset(onesb[:], 1.0)
        zerob = const.tile([128, 512], bf16, tag="zerob")
        nc.vector.memset(zerob[:], 0.0)
        w0sb = const.tile([3, 2], f32, tag="w0sb")
        nc.scalar.dma_start(w0sb[:], w0_d[:])

        wstg = stg.tile([128, 3 * 2 * 1024], f32, tag="stg_w", name="wstg",
                        bufs=1)
        nc.scalar.dma_start(
            wstg[:].rearrange("p (c j u) -> p c j u", c=3, j=2),
            wp_d[:].rearrange("c p j u -> p c j u"))
        Wp8 = xtpp.tile([128, 3 * 2 * 1024], fp8, tag="Wp8")
        nc.vector.tensor_copy(Wp8[:], wstg[:])
        Wp8v = Wp8[:].rearrange("p (c j u) -> p c j u", c=3, j=2)
        stg_cm.__exit__(None, None, None)

        bsb = const.tile([128, 6], f32, tag="bsb")
        nc.scalar.dma_start(bsb[:], bp_d[:])

        # ---------------- big SBUF tensors ----------------
        xT = [[big.tile([128, L], bf16, tag=f"xT{t}_{k}", name=f"xT{t}_{k}")
               for k in range(5)] for t in range(2)]
        xTp = [[xtpp.tile([128 if pc < 2 else 64, 2, L], fp8,
                          tag=f"xTp{t}_{pc}", name=f"xTp{t}_{pc}")
                for pc in range(3)] for t in range(2)]
        for t in range(2):
            nc.gpsimd.memset(xTp[t][2][:, 1, :], 0.0)
        dTP = [dpool.tile([128, 2, L], fp8, tag=f"dTP{t}", name=f"dTP{t}")
               for t in range(2)]
        dT2p = [dpool.tile([32, 2, L], fp8, tag=f"dT2p{t}", name=f"dT2p{t}")
                for t in range(2)]
        for t in range(2):
            nc.gpsimd.memset(dT2p[t][:, 1, :], 0.0)
        nats = [[None] * NP for _ in range(2)]      # main [128,2,512]
        natt = [[None] * NP for _ in range(2)]      # tail [128,2,128]

        # Pool-engine copy helper
        def pool_copy(out, in_):
            nc.gpsimd.tensor_copy(out, in_)

        # ---------------- phase 0: load, transpose, dense ----------------
        def p0_group(g, t):
            pjs2 = [ps.tile([128, 2, 512], bf16, tag="fmp", name="pjs",
                            bufs=4) for _ in range(3)]
            pjs = [pjs2[k // 2][:, k % 2, :] for k in range(5)]
            nf2s = []
            for pp in range(TG // 2):
                pi = g * (TG // 2) + pp
                nf2 = nf32p.tile([128, 2, D], bf16, tag="nf", name="nf2",
                                 bufs=4)
                eng = nc.sync if pp % 2 == 0 else nc.scalar
                eng.dma_start(
                    nf2[:],
                    x_d[t, ds(pi * 256, 256), :].rearrange(
                        "(j k) d -> k j d", j=2))
                nf2s.append((pi, nf2))
                for j in range(2):
                    ii = pp * 2 + j
                    for k, (doff, dcnt) in enumerate(DCH):
                        nc.tensor.transpose(
                            pjs[k][:dcnt, ds(ii * 128, 128)],
                            nf2[:, j, ds(doff, dcnt)], identb[:])
            # evict transposes -> xT (bf16), then xT -> fp8 pairs; chunk
            # pipeline split across DVE/Pool so the dense chain advances
            # on both engines
            gsl = ds(g * GW, GW)
            xtp_dst = [(0, 0, 0, 128), (0, 1, 1, 128), (1, 0, 2, 128),
                       (1, 1, 3, 128), (2, 0, 4, 64)]
            for k, (doff, dcnt) in enumerate(DCH):
                if k in (1, 3):
                    nc.scalar.copy(xT[t][k][:dcnt, gsl], pjs[k][:dcnt, :])
                else:
                    nc.vector.tensor_copy(xT[t][k][:dcnt, gsl],
                                          pjs[k][:dcnt, :])
                pc, j, src, cnt = xtp_dst[k]
                cp = pool_copy if k in (2, 4) else nc.vector.tensor_copy
                cp(xTp[t][pc][:cnt, j, gsl], xT[t][src][:cnt, gsl])
                if k == 4:
                    nc.vector.tensor_copy(xTp[t][2][0:24, 1, gsl],
                                          xT[t][4][64:88, gsl])
            # dense DR matmuls for this column group; evictions are
            # relu+bias via STT on DVE/Pool (keeps ACT a pure exp stream)
            def relu_ev(dst, src, bias, on_act):
                if on_act:
                    nc.scalar.activation(dst, src, AF.Relu, bias=bias)
                else:
                    nc.vector.scalar_tensor_tensor(
                        dst, src, bias, zerob[:src.shape[0], :],
                        op0=ALU.add, op1=ALU.max)
            for m, (uoff, ucnt) in enumerate([(0, 128), (128, 128),
                                              (256, 44)]):
                accd = fmp_t(name="accd")
                for pc in range(3):
                    pp = 128 if pc < 2 else 64
                    nc.tensor.matmul(
                        accd[:ucnt, :],
                        Wp8v[:pp, pc, :, ds(t * U + uoff, ucnt)],
                        xTp[t][pc][:pp, :, gsl],
                        start=(pc == 0), stop=(pc == 2), perf_mode=DR)
                if m < 2:
                    relu_ev(dTP[t][:, m, gsl], accd[:ucnt, :],
                            bsb[:ucnt, t * 3 + m: t * 3 + m + 1],
                            on_act=(m == 1))
                else:
                    relu_ev(dT2p[t][0:32, 0, gsl], accd[0:32, :],
                            bsb[0:32, t * 3 + m: t * 3 + m + 1],
                            on_act=False)
                    relu_ev(dT2p[t][0:12, 1, gsl], accd[32:44, :],
                            bsb[32:44, t * 3 + m: t * 3 + m + 1],
                            on_act=True)
            # nats builds last: off the dense-affinity critical chain
            for pi, nf2 in nf2s:
                nt = natp.tile([128, 2, 512], fp8, tag="nat",
                               name=f"nat{t}_{pi}")
                tl = natp.tile([128, 2, 128], fp8, tag="ntl",
                               name=f"ntl{t}_{pi}")
                nats[t][pi] = nt
                natt[t][pi] = tl
                pool_copy(nt[:], nf2[:, :, 0:512])
                nc.vector.tensor_copy(tl[:, :, 0:88], nf2[:, :, 512:D])
                nc.gpsimd.memset(tl[:, :, 88:ONES_COL], 0.0)
                nc.gpsimd.memset(tl[:, :, ONES_COL + 1:128], 0.0)
                nc.gpsimd.memset(tl[:, :, ONES_COL:ONES_COL + 1], 1.0)

        # ---------------- phase 1: affinity -> E (both layouts) ----------
        def e_tiles(tag):
            return [epool.tile([128, 2, L], fp8, tag="E", name=f"E{tag}_{pi}")
                    for pi in range(NP)]

        def emit_e_unit(a, b, E, i, h):
            """One (l-tile, half) of E = exp(SCALE * dense_a.T @ dense_b)."""
            e = E[i // 2]
            ej = i % 2
            isl = ds(i * 128, 128)
            acc = acc_t(name="eacc")
            for sx in range(2):
                nsl = ds(h * 1024 + sx * 512, 512)
                asl = ds(sx * 512, 512)
                nc.tensor.matmul(acc[:, asl], dTP[a][:, :, isl],
                                 dTP[b][:, :, nsl],
                                 start=True, stop=False, perf_mode=DR)
                nc.tensor.matmul(acc[:, asl], dT2p[a][:, :, isl],
                                 dT2p[b][:, :, nsl],
                                 start=False, stop=True, perf_mode=DR)
            nc.scalar.activation(e[:, ej, ds(h * 1024, 1024)],
                                 acc[:, :], AF.Exp, scale=SCALE)

        # ---------------- aligned + FM per side ----------------
        def aligned_T(s, E, side_tag, hook=None, r_on_act=True,
                      psa_on_acc=False):
            """alT[k] [d, L] bf16 = normalized aligned.T."""
            nat = nats[s]
            ntl = natt[s]
            # pass A: d 512:600 + ones row
            if psa_on_acc:
                psAt = [acc_t(name="psA") for _ in range(2)]
                psA = [psAt[nx // 2][:, ds((nx % 2) * 512, 512)]
                       for nx in range(4)]
            else:
                psA = [fmp_t(name="psA") for _ in range(4)]
            for pi in range(NP):
                for nx in range(4):
                    nc.tensor.matmul(psA[nx][:, :],
                                     ntl[pi][:],
                                     E[pi][:, :, ds(nx * 512, 512)],
                                     start=(pi == 0), stop=(pi == NP - 1),
                                     perf_mode=DR)
            # R chain
            R = big.tile([128, L], bf16, tag="R", name=f"R{side_tag}")
            for h in range(NH):
                rr = rp.tile([128, 1024], f32, tag="rr", name="rr")
                rrb = rp.tile([128, 1024], bf16, tag="rrb", name="rrb")
                for sx in range(2):
                    nc.vector.reciprocal(
                        rr[96:97, ds(sx * 512, 512)],
                        psA[h * 2 + sx][ONES_ROW:ONES_ROW + 1, :])
                nc.vector.tensor_copy(rrb[96:97, :], rr[96:97, :])
                # bc must come from the OTHER psum ring than psA (psA slots
                # are all live until the alT4 eviction, which needs R)
                if psa_on_acc:
                    for sx in range(2):
                        bcx = fmp_t(name="bc")
                        nc.tensor.matmul(bcx[:, :], onesb[96:97, 0:128],
                                         rrb[96:97, ds(sx * 512, 512)],
                                         start=True, stop=True,
                                         tile_position=(96, 0))
                        if r_on_act:
                            nc.scalar.copy(
                                R[:, ds(h * 1024 + sx * 512, 512)], bcx[:, :])
                        else:
                            nc.vector.tensor_copy(
                                R[:, ds(h * 1024 + sx * 512, 512)], bcx[:, :])
                else:
                    bc = acc_t(name="bc")
                    for sx in range(2):
                        nc.tensor.matmul(bc[:, ds(sx * 512, 512)],
                                         onesb[96:97, 0:128],
                                         rrb[96:97, ds(sx * 512, 512)],
                                         start=True, stop=True,
                                         tile_position=(96, 0))
                    if r_on_act:
                        nc.scalar.copy(R[:, ds(h * 1024, 1024)], bc[:, :])
                    else:
                        nc.vector.tensor_copy(R[:, ds(h * 1024, 1024)],
                                              bc[:, :])
            alT = [alp.tile([128, L], bf16, tag=f"alT{k}",
                            name=f"alT{side_tag}{k}") for k in range(5)]
            # evict pass A (d-chunk 4)
            for nx in range(4):
                nsl = ds(nx * 512, 512)
                nc.vector.tensor_mul(alT[4][0:88, nsl], psA[nx][0:88, :],
                                     R[0:88, nsl])
            # passes m=0..3 (hook interleaves independent PE work)
            for m in range(4):
                for h in range(NH):
                    acc = acc_t(name="alacc")
                    for pi in range(NP):
                        for sx in range(2):
                            asl = ds(sx * 512, 512)
                            nsl = ds(h * 1024 + sx * 512, 512)
                            nc.tensor.matmul(acc[:, asl],
                                             nat[pi][:, :, ds(m * 128, 128)],
                                             E[pi][:, :, nsl],
                                             start=(pi == 0),
                                             stop=(pi == NP - 1),
                                             perf_mode=DR)
                    hsl = ds(h * 1024, 1024)
                    nc.vector.tensor_mul(alT[m][:, hsl], acc[:, :], R[:, hsl])
                if hook is not None:
                    hook(m)
            return alT

        def prebuild_b2(s, xTs, on_pool, nk=4):
            """Pair-b tiles (b^2, xb^2) with the b^2 row built early."""
            PBt = []
            for k, (doff, dcnt) in enumerate(DCH[:nk]):
                PB = fmbb.tile([128, 2, L], fp8, tag="PBb", name=f"PBb{s}_{k}")
                b_ = xTs[k][:dcnt, :]
                if on_pool == 'pool':
                    nc.gpsimd.tensor_mul(PB[:dcnt, 0, :], b_, b_)
                elif on_pool == 'dve':
                    nc.vector.tensor_mul(PB[:dcnt, 0, :], b_, b_)
                else:
                    nc.scalar.activation(PB[:dcnt, 0, :], b_, AF.Square)
                PBt.append(PB)
            return PBt

        def fm_side(s, alT, xTs, PBt):
            """FM heads for side s: x = alT (aligned), b = xTs (raw).

            Pair-a = (x^2, xb), pair-b = (b^2, xb^2).  Both DR matmuls
            accumulate into the same base-0 psum rows (disjoint stationary
            columns); X/B bf16 groups are nx-packed at positions 0/32/64/96.
            """
            sk = lambda k: s * 5 + k
            PAt = []
            for k, (doff, dcnt) in enumerate(DCH):
                if k >= len(PBt):
                    PBb = fmbb.tile([128, 2, L], fp8, tag="PBb",
                                    name=f"PBb{s}_{k}")
                    b2_ = xTs[k][:dcnt, :]
                    if s == 0:
                        nc.gpsimd.tensor_mul(PBb[:dcnt, 0, :], b2_, b2_)
                    else:
                        nc.scalar.activation(PBb[:dcnt, 0, :], b2_, AF.Square)
                    PBt.append(PBb)
                PA = fma.tile([128, 2, L], fp8, tag="PAa", name=f"PAa{s}_{k}")
                PBb = PBt[k]
                x_ = alT[k][:dcnt, :]
                b_ = xTs[k][:dcnt, :]
                if s == 0:
                    nc.gpsimd.tensor_mul(PA[:dcnt, 0, :], x_, x_)
                    nc.vector.tensor_mul(PA[:dcnt, 1, :], x_, b_)
                    nc.scalar.activation(PBb[:dcnt, 1, :], PA[:dcnt, 1, :],
                                         AF.Square)
                else:
                    nc.scalar.activation(PA[:dcnt, 0, :], x_, AF.Square)
                    nc.vector.tensor_mul(PA[:dcnt, 1, :], x_, b_)
                    nc.scalar.activation(PBb[:dcnt, 1, :], PA[:dcnt, 1, :],
                                         AF.Square)
                PAt.append(PA)
            # projections: AB nx-packed in 2 fmp tiles; pair-a/pair-b DR
            # into base-0 rows of 2 acc tiles (one 512-half per nx)
            ABt = [fmp_t(name=f"ABt{i}") for i in range(2)]
            BC = [acc_t(name=f"BC{i}") for i in range(2)]
            for k, (doff, dcnt) in enumerate(DCH):
                fl = (k == 0, k == 4)
                for nx in range(4):
                    nsl = ds(nx * 512, 512)
                    AB = ABt[nx // 2]
                    pb = (nx % 2) * 64
                    nc.tensor.matmul(AB[pb:pb + 32, :],
                                     pa_stat[s][k][:dcnt, 0:32],
                                     alT[k][:dcnt, nsl],
                                     start=fl[0], stop=fl[1],
                                     tile_position=(0, pb),
                                     skip_group_check=True)
                    nc.tensor.matmul(AB[pb + 32:pb + 64, :],
                                     pa_stat[s][k][:dcnt, 32:64],
                                     xTs[k][:dcnt, nsl],
                                     start=fl[0], stop=fl[1],
                                     tile_position=(0, pb + 32),
                                     skip_group_check=True)
                    hsl = ds((nx % 2) * 512, 512)
                    nc.tensor.matmul(BC[nx // 2][0:32, hsl],
                                     PBCv[:dcnt, sk(k), :, 0:32],
                                     PAt[k][:dcnt, :, nsl],
                                     start=fl[0], stop=False,
                                     perf_mode=DR, skip_group_check=True)
                    nc.tensor.matmul(BC[nx // 2][0:32, hsl],
                                     PBCv[:dcnt, sk(k), :, 32:64],
                                     PBt[k][:dcnt, :, nsl],
                                     start=False, stop=fl[1],
                                     perf_mode=DR, skip_group_check=True)
            # combine
            S = sp.tile([128, L], bf16, tag="S", name=f"S{s}")
            TAs = sp.tile([16, L], bf16, tag="TAs", name=f"TAs{s}")
            for nx in range(4):
                nsl = ds(nx * 512, 512)
                AB = ABt[nx // 2]
                pb = (nx % 2) * 64
                hsl = ds((nx % 2) * 512, 512)
                nc.vector.tensor_copy(S[0:64, nsl], AB[pb:pb + 64, :])
                nc.vector.tensor_copy(S[64:96, nsl], BC[nx // 2][0:32, hsl])
                nc.vector.tensor_add(TAs[0:10, nsl], AB[pb:pb + 10, :],
                                     S[32:42, nsl])
                # M^2 -> S[96:101] (outside the copied range): psum x the
                # raw bf16 copy in S[64:69] (only one PSUM input allowed)
                if s == 0:
                    nc.vector.tensor_mul(S[96:101, nsl],
                                         BC[nx // 2][0:5, hsl],
                                         S[64:69, nsl])
                else:
                    nc.scalar.activation(S[96:101, nsl],
                                         BC[nx // 2][0:5, hsl], AF.Square)
            if s == 0:
                nc.vector.tensor_mul(S[0:10, :], TAs[0:10, :], TAs[0:10, :])
            else:
                nc.scalar.activation(S[0:10, :], TAs[0:10, :], AF.Square)
            for nx in range(4):
                nsl = ds(nx * 512, 512)
                cps = fmp_t(name="cps")
                nc.tensor.matmul(cps[0:3, :], cb2[0:101, ds(s * 3, 3)],
                                 S[0:101, nsl], start=True, stop=True)
                o = ob.tile([3, 512], f32, tag="ob", name="o")
                if s == 0:
                    nc.vector.scalar_tensor_tensor(
                        o[:, :], cps[0:3, :], w0sb[:, s:s + 1],
                        zerob[0:3, :], op0=ALU.add, op1=ALU.add)
                else:
                    nc.scalar.activation(o[:, :], cps[0:3, :], AF.Identity,
                                         bias=w0sb[:, s:s + 1])
                nc.sync.dma_start(out_d[s, :, nsl], o[:, :])

        # ---------------- main flow ----------------
        epool = ctx.enter_context(tc.tile_pool(name="epool", bufs=LT))
        E1 = e_tiles("1")
        E2 = e_tiles("2")

        def e_batch(g):
            # all affinity units whose dense column groups are complete
            units = []
            if g >= 1:
                hs = [0] if g < 3 else [0, 1]
                for h in hs:
                    if h == 1:
                        iset = range(LT)
                    else:
                        iset = (range((g + 1) * TG) if g < 3
                                else range(2 * TG, LT))
                    for i in iset:
                        units.append((i, h))
            # E1 only (aligned1 is the nearest consumer); E2 units are
            # deferred so the ACT queue drains all E1 exps first.
            for i, h in units:
                if (i, h) not in emitted1:
                    emitted1.add((i, h))
                    emit_e_unit(0, 1, E1, i, h)
                    deferred2.append((i, h))

        emitted1 = set()
        deferred2 = []
        for g in range(NG):
            p0_group(g, 0)
            p0_group(g, 1)
            e_batch(g)

        lstg_cm = tc.tile_pool(name="lstg", bufs=1, side="right")
        lstg = lstg_cm.__enter__()
        pstg = lstg.tile([128, 10 * 64], f32, tag="stg_pa", name="pstg",
                         bufs=1)
        nc.scalar.dma_start(
            pstg[:].rearrange("p (t c) -> p t c", t=10),
            pa_d[:].rearrange("t p c -> p t c"))
        PAst = const.tile([128, 10 * 64], bf16, tag="PAst")
        nc.vector.tensor_copy(PAst[:], pstg[:])
        pa_stat = [[PAst[:, ds((s * 5 + k) * 64, 64)] for k in range(5)]
                   for s in range(2)]
        bstg = lstg.tile([128, 10 * 2 * 64], f32, tag="stg_pbc", name="bstg",
                         bufs=1)
        nc.scalar.dma_start(
            bstg[:].rearrange("p (t j c) -> p t j c", t=10, j=2),
            pbc_d[:].rearrange("t p j c -> p t j c"))
        PBCst = const.tile([128, 10 * 2 * 64], fp8, tag="PBCst")
        nc.vector.tensor_copy(PBCst[:], bstg[:])
        PBCv = PBCst[:].rearrange("p (t j c) -> p t j c", t=10, j=2)
        cstg = lstg.tile([128, 6], f32, tag="stg_c", name="cstg", bufs=1)
        nc.scalar.dma_start(cstg[:], c2_d[:])
        cb2 = const.tile([128, 6], bf16, tag="cb2")
        nc.vector.tensor_copy(cb2[:], cstg[:])
        lstg_cm.__exit__(None, None, None)
        nf32_cm.__exit__(None, None, None)
        xtp_cm.__exit__(None, None, None)
        alp = ctx.enter_context(tc.tile_pool(name="alp", bufs=1))
        rp = ctx.enter_context(tc.tile_pool(name="rp", bufs=1))

        # E2 units in tile-major order, interleaved into aligned1's m-loop
        # (the PSUM acc ring is allocation-ordered, so emitting them after
        # each m-pass lets aligned1 run as soon as E1 is ready while E2
        # trails on ACT).
        e2units = [(2 * pi + j, h)
                   for pi in range(NP) for j in range(2) for h in range(NH)]

        def e2_hook(m):
            for i, h in e2units[m * 8:(m + 1) * 8]:
                emit_e_unit(1, 0, E2, i, h)

        fma = ctx.enter_context(tc.tile_pool(name="fma", bufs=3))
        fmbb = ctx.enter_context(tc.tile_pool(name="fmbb", bufs=5))
        # side-0 b^2 prebuilt on Pool (fills the E1-exp window)
        PB0 = prebuild_b2(0, xT[0], on_pool='dve')
        qaT = aligned_T(1, E1, "q", hook=e2_hook, r_on_act=False)
        dpool_cm.__exit__(None, None, None)
        sp = ctx.enter_context(tc.tile_pool(name="sp", bufs=1))
        ob = ctx.enter_context(tc.tile_pool(name="ob", bufs=4))
        fm_side(0, qaT, xT[0], PB0)
        PB1 = prebuild_b2(1, xT[1], on_pool='act')
        paT = aligned_T(0, E2, "p", psa_on_acc=True)
        fm_side(1, paT, xT[1], PB1)


def _host_prep(W1, b1, W2, b2, cat_w0, cat_w, cat_V, dm_w0, dm_w, dm_V):
    # dense pair weights: wpair[pc][k][j] = W_t rows; pc<2: d=pc*256+j*128+k
    # pc=2 (44 partitions): d = 512 + j*44 + k
    wpair = np.zeros((3, 128, 2, 1024), np.float32)
    for t, W in enumerate((W1, W2)):
        for pc in range(2):
            for j in range(2):
                d0 = pc * 256 + j * 128
                wpair[pc, :, j, t * U:(t + 1) * U] = W[d0:d0 + 128]
        wpair[2, 0:64, 0, t * U:(t + 1) * U] = W[512:576]
        wpair[2, 0:24, 1, t * U:(t + 1) * U] = W[576:600]

    # PA stationaries (bf16): cols 0:12 x-side, 32:44 b-side (padded to 32/64)
    pastat = np.zeros((10, 128, 64), np.float32)
    # PB/PC stationaries (fp8 pairs)
    pbcstat = np.zeros((10, 128, 2, 64), np.float32)
    for s in range(2):
        ci, di, mi = s, s, s + 2
        Va = cat_V[ci][:, :D]
        Vb = cat_V[ci][:, D:]
        Vd = dm_V[di]
        Vm = dm_V[mi]
        ua = (Va ** 2).sum(0) * USC
        ub = (Vb ** 2).sum(0) * USC
        ud = (Vd ** 2).sum(0) * USC
        um = (Vm ** 2).sum(0) * USC
        xs = np.zeros((D, 64), np.float32)
        xs[:, 0:5] = Va.T
        xs[:, 5:10] = Vd.T
        xs[:, 10] = cat_w[ci, :D]
        xs[:, 11] = dm_w[di]
        xs[:, 32 + 0:32 + 5] = Vb.T
        xs[:, 32 + 5:32 + 10] = -Vd.T
        xs[:, 32 + 10] = cat_w[ci, D:]
        xs[:, 32 + 11] = dm_w[di]
        bs = np.zeros((D, 2, 64), np.float32)
        # pair-a = (x^2, xb): j0 -> x^2 stats, j1 -> xb stats
        bs[:, 0, 5] = ua
        bs[:, 0, 6] = ud
        bs[:, 1, 0:5] = Vm.T
        bs[:, 1, 7] = dm_w[mi]
        bs[:, 1, 8] = ud
        # pair-b = (b^2, xb^2): j0 -> b^2 stats, j1 -> xb^2 stats
        bs[:, 0, 32 + 9] = ub
        bs[:, 0, 32 + 10] = ud
        bs[:, 1, 32 + 11] = um
        for k, (doff, dcnt) in enumerate(DCH):
            pastat[s * 5 + k, :dcnt] = xs[doff:doff + dcnt]
            pbcstat[s * 5 + k, :dcnt] = bs[doff:doff + dcnt]

    # combine matrix: S rows -> 3 outputs per side
    comb2 = np.zeros((128, 6), np.float32)
    for s in range(2):
        C = comb2[:, s * 3:(s + 1) * 3]
        C[10, 0] = 1.0          # x@w_cat
        C[42, 0] = 1.0          # b@w_cat2
        C[0:5, 0] = 0.5         # cat quads (TA^2 rows)
        C[69, 0] = -0.5 / USC   # x2@ua
        C[73, 0] = -0.5 / USC   # b2@ub
        C[11, 1] = 1.0          # x@w_d
        C[43, 1] = -1.0         # -b@w_d
        C[5:10, 1] = 0.5        # diff quads (TA^2 rows)
        C[70, 1] = -0.5 / USC   # x2@ud
        C[74, 1] = -0.5 / USC   # b2@ud
        C[72, 1] = 1.0 / USC    # xb@ud
        C[71, 2] = 1.0          # xb@w_m
        C[96:101, 2] = 0.5      # mul quads (M^2 rows)
        C[75, 2] = -0.5 / USC   # xb2@um

    biasp = np.zeros((128, 6), np.float32)
    for t, b in enumerate((b1, b2)):
        for m, (uoff, ucnt) in enumerate([(0, 128), (128, 128), (256, 44)]):
            if m < 2:
                biasp[:ucnt, t * 3 + m] = b[uoff:uoff + ucnt]
            else:
                biasp[0:32, t * 3 + m] = b[256:288]
                biasp[32:44, t * 3 + m] = b[288:300]

    w0col = np.zeros((3, 2), np.float32)
    for s in range(2):
        w0col[0, s] = cat_w0[s, 0]
        w0col[1, s] = dm_w0[s, 0]
        w0col[2, s] = dm_w0[s + 2, 0]
    return wpair, pastat, pbcstat, comb2, biasp, w0col


_PROG = None


def _get_prog():
    global _PROG
    if _PROG is None:
        from concourse import bacc
        nc = bacc.Bacc(None, target_bir_lowering=False)
        _emit(nc, L_FULL)
        nc.finalize()
        _PROG = nc
    return _PROG


def _in_maps(stack_input, W1, b1, W2, b2, fm_cat_w0, fm_cat_w, fm_cat_V,
             fm_dm_w0, fm_dm_w, fm_dm_V):
    f = lambda a: np.ascontiguousarray(np.asarray(a, np.float32))
    stack_input = f(stack_input)
    wpair, pastat, pbcstat, comb2, biasp, w0col = _host_prep(
        f(W1), f(b1), f(W2), f(b2), f(fm_cat_w0), f(fm_cat_w), f(fm_cat_V),
        f(fm_dm_w0), f(fm_dm_w), f(fm_dm_V))
    import ml_dtypes
    common = {"wpair": wpair, "pastat": pastat, "pbcstat": pbcstat,
              "comb2": comb2, "biasp": biasp, "w0col": w0col}
    xb = np.ascontiguousarray(stack_input.astype(ml_dtypes.bfloat16))
    return [dict(common, x=np.ascontiguousarray(xb[:, b]))
            for b in range(N_CORES)]


def kernel(stack_input, W1, b1, W2, b2, fm_cat_w0, fm_cat_w, fm_cat_V,
           fm_dm_w0, fm_dm_w, fm_dm_V):
    from concourse.bass_utils import run_bass_kernel_spmd

    in_maps = _in_maps(stack_input, W1, b1, W2, b2, fm_cat_w0, fm_cat_w,
                       fm_cat_V, fm_dm_w0, fm_dm_w, fm_dm_V)
    nc = _get_prog()
    res = run_bass_kernel_spmd(nc, in_maps, core_ids=list(range(N_CORES)))
    outs = [r["out"] for r in res.results]            # each [2, 3, L]
    fp = np.stack([o[0].T for o in outs]).astype(np.float32)   # [8, L, 3]
    fq = np.stack([o[1].T for o in outs]).astype(np.float32)
    return fp, fq

